# revision 8
# baseline (speedup 1.0000x reference)
"""Trainium2 Bass kernel for nn_Kernel_12281761695451725822_53472342835843.

Computation (per sample n, channel c):
  t3 = Conv1x5(x, w3)                      (channel-mixing 1x5 conv, pad 2)
  t7 = sum over 9 (oh,ow) terms of w7[c,3*ow+oh] * max(A_ohow, B_ohow)
       A = x[h, w+2*oh-2] (zero pad), B = x[h+oh-1, w+2*ow-3] (zero pad; w=0 wraps)
  out = t7 * t3

Strategy (pure data-parallel over batch: 2 samples per core, 8 cores):
  - SBUF layout: 128 partitions = (n_local, c); free dim = padded canvas (h, w).
  - The 9 max terms reduce to 6 shared pair-max tensors M_delta (max is symmetric).
  - DVE computes the 6 maxes in fp16 (2x mode); even alignment via a 1-shifted
    canvas copy so both operands of every max are 4B-aligned.
  - PE (fp16 matmuls, 1 cycle/row) computes t3 (5 block-diag taps) and the 9
    weighted tap accumulations of t7 (diagonal lhsT), accumulating in PSUM.
  - ACT drains both PSUM accumulators to SBUF fp16; DVE does the final t7*t3.
  - w=0 output column is recomputed exactly (roll-by-1 wrap) and patched in.
"""

import numpy as np

N, C, H, W = 16, 64, 128, 128
NCORES = 8
NLOC = N // NCORES          # samples per core
P = 128                     # partitions = NLOC * C
ROWS = H + 2                # canvas rows, storage row = h + 1
COLS = 144                  # canvas cols, storage col = u + UOFF, u in [-9, 135)
UOFF = 9
HS = 8                      # strip height (output rows per strip)
NSTRIPS = H // HS
OUT_ROWS = 16               # output staging rows per DMA
MCOLS = 136                 # M tensor storage stride (cols), valid [0, 134)

# term i: (oh, ow, (dh, dw) of shared max, read offset (dr, du) into M)
TERMS = [
    (0, 0, (1, 1), -1, -3),
    (0, 1, (1, -1), -1, -1),
    (0, 2, (1, -3), -1, 1),
    (1, 0, (0, 3), 0, -3),
    (1, 1, (0, 1), 0, -1),
    (1, 2, (0, 1), 0, 0),
    (2, 0, (1, -5), 0, 2),
    (2, 1, (1, -3), 0, 2),
    (2, 2, (1, -1), 0, 2),
]
DELTAS = [(1, 1), (1, -1), (1, -3), (0, 3), (0, 1), (1, -5)]


def build_host_weights(w3, w7):
    """Host-side packing of the conv weights into PE lhsT layouts (fp16)."""
    w3 = np.asarray(w3, dtype=np.float32)
    w7 = np.asarray(w7, dtype=np.float32)
    wt3 = np.zeros((5, P, P), dtype=np.float16)
    for k in range(5):
        blk = w3[:, :, 0, k].T.astype(np.float16)  # [ci, co]
        for n in range(NLOC):
            wt3[k, n * C:(n + 1) * C, n * C:(n + 1) * C] = blk
    wt7 = np.zeros((9, P, P), dtype=np.float16)
    w7c = np.zeros((P, 9), dtype=np.float16)
    for i, (oh, ow, _d, _dr, _du) in enumerate(TERMS):
        kidx = 3 * ow + oh
        vals = np.concatenate([w7[:, kidx], w7[:, kidx]]).astype(np.float16)  # [P]
        wt7[i, np.arange(P), np.arange(P)] = vals
        w7c[:, i] = vals
    return wt3, wt7, w7c


def build_program():
    """Build and compile the single-core Bass/Tile program (SPMD-replicated)."""
    import concourse.bacc as bacc
    import concourse.tile as tile
    import concourse.mybir as mybir

    fp16 = mybir.dt.float16
    fp32 = mybir.dt.float32
    AOT = mybir.AluOpType

    nc = bacc.Bacc("TRN2", target_bir_lowering=False, debug=False,
                   enable_asserts=False, num_devices=1)
    x_in = nc.dram_tensor("x_in", [P, H, W], fp32, kind="ExternalInput")
    wt3_d = nc.dram_tensor("wt3", [5, P, P], fp16, kind="ExternalInput")
    wt7_d = nc.dram_tensor("wt7", [9, P, P], fp16, kind="ExternalInput")
    w7c_d = nc.dram_tensor("w7c", [P, 9], fp16, kind="ExternalInput")
    out_d = nc.dram_tensor("out", [P, H, W], fp32, kind="ExternalOutput")

    with tile.TileContext(nc) as tc:
        with (
            tc.tile_pool(name="persist", bufs=1) as persist,
            tc.tile_pool(name="mpool", bufs=2) as mpool,
            tc.tile_pool(name="ppool", bufs=2, space="PSUM") as ppool,
            tc.tile_pool(name="spool", bufs=3) as spool,
            tc.tile_pool(name="opool", bufs=2) as opool,
            tc.tile_pool(name="fixp", bufs=2) as fixp,
        ):
            canvas = persist.tile([P, ROWS, COLS], fp16, tag="canvas")
            canvaso = persist.tile([P, ROWS, COLS], fp16, tag="canvaso")
            wt3_s = persist.tile([P, 5, P], fp16, tag="wt3")
            wt7_s = persist.tile([P, 9, P], fp16, tag="wt7")
            w7c_s = persist.tile([P, 9], fp16, tag="w7c")
            t7c0 = persist.tile([P, H], fp16, tag="t7c0")

            # weights in
            nc.sync.dma_start(out=wt3_s, in_=wt3_d.ap().rearrange("k a b -> a k b"))
            nc.sync.dma_start(out=wt7_s, in_=wt7_d.ap().rearrange("k a b -> a k b"))
            nc.sync.dma_start(out=w7c_s, in_=w7c_d.ap())

            # canvas: zero borders, then stream x in 32-row chunks so strip
            # compute overlaps the input pipeline: dense cast-DMA to staging,
            # ACT copy into the padded interior, flat-shift DMA for canvaso.
            stgt = persist.tile([P, H, W], fp16, tag="stg")
            nc.vector.memset(canvas[:, 0, :], 0.0)
            nc.vector.memset(canvas[:, H + 1, :], 0.0)
            nc.vector.memset(canvas[:, 1:H + 1, 0:UOFF], 0.0)
            nc.vector.memset(canvas[:, 1:H + 1, UOFF + W:COLS], 0.0)
            cflat = canvas.rearrange("p r c -> p (r c)")
            coflat = canvaso.rearrange("p r c -> p (r c)")
            CH = 32
            for cb in range(H // CH):
                h0 = cb * CH
                nc.gpsimd.dma_start(out=stgt[:, h0:h0 + CH, :],
                                    in_=x_in.ap()[:, h0:h0 + CH, :])
                nc.scalar.copy(out=canvas[:, 1 + h0:1 + h0 + CH, UOFF:UOFF + W],
                               in_=stgt[:, h0:h0 + CH, :])
                # canvaso rows [h0 .. h0+CH): flat shift; the chunk's last
                # element reads canvas[r_next, 0], a zeroed border column.
                f0 = (1 + h0) * COLS if cb > 0 else 0
                f1 = (1 + h0 + CH) * COLS if cb < H // CH - 1 else ROWS * COLS - 1
                nc.gpsimd.dma_start(out=coflat[:, f0:f1],
                                    in_=cflat[:, f0 + 1:f1 + 1])

            # --- w=0 column fixup values: t7c0[p, h] = sum_i w_i * max(A_i, B_i) at w=0
            for i, (oh, ow, _d, _dr, _du) in enumerate(TERMS):
                tmp = fixp.tile([P, H], fp16, tag="fixtmp")
                a_ap = canvas[:, 1:1 + H, 2 * oh + 7]
                b_ap = canvas[:, oh:oh + H, 134 + 2 * ow]
                nc.vector.tensor_tensor(tmp, a_ap, b_ap, AOT.max)
                nc.vector.scalar_tensor_tensor(
                    out=t7c0, in0=tmp, scalar=w7c_s[:, i:i + 1],
                    in1=(tmp if i == 0 else t7c0),
                    op0=AOT.mult, op1=(AOT.bypass if i == 0 else AOT.add))

            # --- main strip loop
            for s in range(NSTRIPS):
                r0 = s * HS
                # 6 shared max tensors for this strip (rows r0-1 .. r0+HS-1)
                mts = {}
                for di, (dh, dw) in enumerate(DELTAS):
                    mt = mpool.tile([P, HS + 1, MCOLS], fp16, tag=f"m{di}")
                    in0 = canvas[:, r0:r0 + HS + 1, 6:140]
                    v0 = 5 + dw
                    in1 = canvaso[:, r0 + dh:r0 + dh + HS + 1, v0:v0 + 134]
                    nc.vector.tensor_tensor(mt[:, :, 0:134], in0, in1, AOT.max)
                    mts[(dh, dw)] = mt

                # t3: 5 block-diag conv taps accumulated in PSUM
                t3p = ppool.tile([P, HS * W], fp32, tag="t3p")
                for k in range(5):
                    for half in range(2):
                        rhs = canvas[:, 1 + r0 + 4 * half:1 + r0 + 4 * half + 4,
                                     7 + k:7 + k + W]
                        nc.tensor.matmul(
                            out=t3p[:, 512 * half:512 * half + 512],
                            lhsT=wt3_s[:, k, :], rhs=rhs,
                            start=(k == 0), stop=(k == 4))

                # t7: 9 weighted tap accumulations in PSUM
                t7p = ppool.tile([P, HS * W], fp32, tag="t7p")
                for i, (_oh, _ow, d, dr, du) in enumerate(TERMS):
                    mt = mts[d]
                    for half in range(2):
                        rhs = mt[:, 4 * half + dr + 1:4 * half + dr + 1 + 4,
                                 du + 3:du + 3 + W]
                        nc.tensor.matmul(
                            out=t7p[:, 512 * half:512 * half + 512],
                            lhsT=wt7_s[:, i, :], rhs=rhs,
                            start=(i == 0), stop=(i == 8))

                # drain PSUM -> SBUF fp16 (ACT engine)
                t3s = spool.tile([P, HS, W], fp16, tag="t3s")
                t7s = spool.tile([P, HS, W], fp16, tag="t7s")
                nc.scalar.copy(out=t3s.rearrange("p a b -> p (a b)"), in_=t3p)
                nc.scalar.copy(out=t7s.rearrange("p a b -> p (a b)"), in_=t7p)

                # patch the wrap column (w=0) of t7
                nc.vector.tensor_copy(t7s[:, :, 0], t7c0[:, r0:r0 + HS])

                # final product into the output staging buffer
                if s % (OUT_ROWS // HS) == 0:
                    outs = opool.tile([P, OUT_ROWS, W], fp16, tag="outs")
                sub = s % (OUT_ROWS // HS)
                nc.vector.tensor_tensor(
                    outs[:, sub * HS:(sub + 1) * HS, :], t7s, t3s, AOT.mult)

                if sub == OUT_ROWS // HS - 1:
                    ro = (s // (OUT_ROWS // HS)) * OUT_ROWS
                    nc.gpsimd.dma_start(out=out_d.ap()[:, ro:ro + OUT_ROWS, :],
                                        in_=outs)

    nc.compile()
    return nc


_PROGRAM = None


def _get_program():
    global _PROGRAM
    if _PROGRAM is None:
        _PROGRAM = build_program()
    return _PROGRAM


def make_in_maps(inputs):
    x = np.asarray(inputs["x"], dtype=np.float32)
    wt3, wt7, w7c = build_host_weights(inputs["w3"], inputs["w7"])
    in_maps = []
    for core in range(NCORES):
        shard = x[core * NLOC:(core + 1) * NLOC].reshape(P, H, W)
        in_maps.append({"x_in": np.ascontiguousarray(shard),
                        "wt3": wt3, "wt7": wt7, "w7c": w7c})
    return in_maps


def kernel(**inputs) -> np.ndarray:
    from concourse.bass_utils import run_bass_kernel_spmd
    nc = _get_program()
    in_maps = make_in_maps(inputs)
    res = run_bass_kernel_spmd(nc, in_maps, core_ids=list(range(NCORES)))
    out = np.empty((N, C, H, W), dtype=np.float32)
    for core in range(NCORES):
        out[core * NLOC:(core + 1) * NLOC] = res.results[core]["out"].reshape(
            NLOC, C, H, W)
    return out
